# revision 1
# baseline (speedup 1.0000x reference)
"""Block-diagonal GRU cell on 8 TRN2 NeuronCores — one block per core.

Math per block n (torch GRUCell):
  gi = x_n @ W_ih[n].T + b_ih[n]        (B, 3*BS)
  gh = h_n @ W_hh[n].T + b_hh[n]
  r = sigmoid(gi_r + gh_r); z = sigmoid(gi_z + gh_z)
  ng = tanh(gi_n + r * gh_n)
  h' = ng + z * (h_n - ng)

On-chip layout (per core): everything transposed on host so the
contraction (feature) dim is the SBUF partition dim and gates land on
PSUM partitions — biases then apply as per-partition ACT/DVE operands.
  A  = [W_ih[n].T ; W_hh[n].T]  -> (1024 feat, 1536 gates), blocked per
       128-gate column group so group DMAs are contiguous.
  U  = [x_n.T ; h_n.T]          -> (1024 feat, 1024 batch)
  out = h'.T                    -> (512, 1024), un-transposed on host.
r/z gates accumulate x- and h-matmuls into one PSUM bank (8 k-steps);
the n gate keeps i_n / h_n in separate banks. Matmuls run as float32r
(E8M11, full-rate fp32 PE mode; host pre-rounds operands to the fp32r
grid). Per output row-block j the r/z/n matmul groups are interleaved
so each combine chain overlaps the next group's matmuls; bulk loads are
a few large DMAs on one HWDGE queue (Sync), emitted in exact
consumption order, with dummy PE warm-up matmuls bridging the fill.
"""

import os
import sys

import numpy as np

try:
    import concourse.bass as bass
except ImportError:  # fresh grading dir: fall back to the repo checkout
    sys.path.insert(0, "/opt/trn_rl_repo")
    import concourse.bass as bass

import concourse.mybir as mybir
import concourse.tile as tile
from concourse import bacc
from concourse.bass import ts
from concourse.bass_utils import run_bass_kernel_spmd

B = 1024            # batch
NB = 8              # blocks == cores
BS = 512            # hidden block size
G3 = 3 * BS         # gates per block (r, z, n)
KF = 1024           # contraction feats per core: 512 input + 512 hidden
P = 128
KT = KF // P        # 8 k-tiles
GT = G3 // P        # 12 gate column groups: 0-3 r, 4-7 z, 8-11 n
NBC = 2             # batch chunks
BC = B // NBC       # 512 (one PSUM bank of fp32)

F32 = mybir.dt.float32
F32R = mybir.dt.float32r
AFT = mybir.ActivationFunctionType
ALU = mybir.AluOpType

_cache: dict = {}
LAST_RESULTS = None  # BassKernelResults of the most recent run (for test.py)


def _build_nc():
    nc = bacc.Bacc("TRN2", target_bir_lowering=False, debug=False, num_devices=NB)
    a_d = nc.dram_tensor("a", [GT, P, KT, P], F32R, kind="ExternalInput").ap()
    u_d = nc.dram_tensor("u", [KT, P, B], F32R, kind="ExternalInput").ap()
    brz_d = nc.dram_tensor("brz", [P, 12], F32, kind="ExternalInput").ap()
    bn_d = nc.dram_tensor("bn", [P, 8], F32, kind="ExternalInput").ap()
    o_d = nc.dram_tensor("o", [BS, B], F32, kind="ExternalOutput").ap()

    with tile.TileContext(nc) as tc:
        with (
            tc.tile_pool(name="persist", bufs=1) as persist,
            tc.tile_pool(name="tmp", bufs=3) as tmp,
            tc.tile_pool(name="outp", bufs=4) as outp,
            tc.tile_pool(name="psum", bufs=8, space="PSUM") as psum,
        ):
            # small bias loads ride the gpsimd SWDGE queue, off the bulk path
            brz_sb = persist.tile([P, 12], F32, name="brz_sb")
            nc.gpsimd.dma_start(brz_sb[:], brz_d[:])
            bn_sb = persist.tile([P, 8], F32, name="bn_sb")
            nc.gpsimd.dma_start(bn_sb[:], bn_d[:])

            # Bulk loads: one HWDGE queue (Sync), strict consumption order.
            # Host lays A out in per-j slots [r_j, z_j, n_j] so per-slot
            # DMAs arrive exactly as the matmul groups consume them.
            U = persist.tile([P, KT, B], F32R, name="U")
            A = persist.tile([P, GT * KT, P], F32R, name="A")

            def load_a(s):
                nc.sync.dma_start(A[:, s * KT : (s + 1) * KT, :], a_d[s])

            def load_u(k0, k1, bc):
                nc.sync.dma_start(
                    U[:, k0:k1, ts(bc, BC)],
                    u_d[k0:k1].rearrange("k p b -> p k b")[:, :, ts(bc, BC)],
                )

            load_u(0, 4, 0)
            load_a(0)
            load_a(1)
            load_u(4, 8, 0)
            for s in range(2, 9):
                load_a(s)
            load_u(0, 8, 1)
            for s in range(9, GT):
                load_a(s)

            # PE warm-up: dummy matmuls on scratch during the DMA fill keep
            # the HAM activity window busy so real matmuls start at 2.4 GHz
            wsb = persist.tile([P, BC], mybir.dt.bfloat16, name="wsb")
            nc.gpsimd.memset(wsb[:], 0.0)
            wps = psum.tile([P, BC], F32, name="wps", tag="ps")
            for _ in range(16):
                nc.tensor.matmul(wps[:], wsb[:, :P], wsb[:], start=True, stop=True)

            # logical gate group -> A slot: slot 3j=r_j (g=j), 3j+1=z_j
            # (g=4+j), 3j+2=n_j (g=8+j)
            def slot_of(g):
                j, kind = g % 4, g // 4
                return 3 * j + kind

            def lhsT(g, k):
                return A[:, slot_of(g) * KT + k, :]

            # persistent per row-block j: r gate, omz = 1-z, zh = z*h
            r_t = [persist.tile([P, B], F32, name=f"r{j}") for j in range(4)]
            omz = [persist.tile([P, B], F32, name=f"omz{j}") for j in range(4)]
            zh = [persist.tile([P, B], F32, name=f"zh{j}") for j in range(4)]

            def mm_group(g, c0, w, k0, k1):
                ps = psum.tile([P, w], F32, name="ps", tag="ps")
                for k in range(k0, k1):
                    nc.tensor.matmul(
                        ps[:],
                        lhsT(g, k),
                        U[:, k, c0 : c0 + w],
                        start=(k == k0),
                        stop=(k == k1 - 1),
                    )
                return ps

            def combine(j, c0, w, ps_i, ps_h, sl):
                # h' = omz*ng + zh, ng = tanh(i_n + b_in + r*(h_n + b_hn))
                t = tmp.tile([P, w], F32, name="t", tag="t")
                nc.vector.scalar_tensor_tensor(
                    t[:], ps_h[:, sl], bn_sb[:, 4 + j : 5 + j],
                    r_t[j][:, c0 : c0 + w], ALU.add, ALU.mult,
                )
                t2 = tmp.tile([P, w], F32, name="t2", tag="t2")
                nc.vector.tensor_add(t2[:], t[:], ps_i[:, sl])
                nt = tmp.tile([P, w], F32, name="nt", tag="nt")
                nc.scalar.activation(nt[:], t2[:], AFT.Tanh, bias=bn_sb[:, j : j + 1])
                m = tmp.tile([P, w], F32, name="m", tag="m")
                nc.vector.tensor_mul(m[:], omz[j][:, c0 : c0 + w], nt[:])
                o_t = outp.tile([P, w], F32, name="o_t", tag="o_t")
                nc.vector.tensor_add(o_t[:], m[:], zh[j][:, c0 : c0 + w])
                nc.sync.dma_start(o_d[ts(j, P), c0 : c0 + w], o_t[:])

            for bc in range(NBC):
                for j in range(4):
                    ps_r = mm_group(j, bc * BC, BC, 0, KT)
                    nc.scalar.activation(
                        r_t[j][:, ts(bc, BC)], ps_r[:], AFT.Sigmoid,
                        bias=brz_sb[:, j : j + 1],
                    )
                    ps_z = mm_group(4 + j, bc * BC, BC, 0, KT)
                    zt = tmp.tile([P, BC], F32, name="zt", tag="zt")
                    nc.scalar.activation(
                        zt[:], ps_z[:], AFT.Sigmoid, bias=brz_sb[:, 4 + j : 5 + j]
                    )
                    # 1 - sigmoid(x) == sigmoid(-x); bias col 8+j holds -b_z
                    nc.scalar.activation(
                        omz[j][:, ts(bc, BC)], ps_z[:], AFT.Sigmoid,
                        bias=brz_sb[:, 8 + j : 9 + j], scale=-1.0,
                    )
                    nc.vector.tensor_mul(
                        zh[j][:, ts(bc, BC)], zt[:],
                        U[:, 4 + j, ts(bc, BC)].bitcast(F32),
                    )
                    if bc == NBC - 1 and j == 3:
                        # final group: half-width n-gate psums so only one
                        # short 256-wide combine chain trails the last matmul
                        HW_ = BC // 2
                        for s in range(2):
                            c0 = bc * BC + s * HW_
                            ps_h = mm_group(8 + j, c0, HW_, 4, KT)
                            ps_i = mm_group(8 + j, c0, HW_, 0, 4)
                            combine(j, c0, HW_, ps_i, ps_h, slice(0, HW_))
                    else:
                        ps_h = mm_group(8 + j, bc * BC, BC, 4, KT)
                        ps_i = mm_group(8 + j, bc * BC, BC, 0, 4)
                        combine(j, bc * BC, BC, ps_i, ps_h, slice(0, BC))

    nc.compile()
    return nc


def _round_fp32r(a):
    """Round fp32 to the fp32r grid (E8M11: low 12 mantissa bits zero, RNE)."""
    b = np.ascontiguousarray(a, dtype=np.float32).view(np.uint32)
    lsb = (b >> 12) & 1
    out = ((b + 0x7FF + lsb) & np.uint32(0xFFFFF000)).view(np.float32)
    return out


_SLOT_TO_G = [g for j in range(4) for g in (j, 4 + j, 8 + j)]


def _prep_core_inputs(x, h, W_ih, W_hh, b_ih, b_hh, n):
    a_full = np.concatenate([W_ih[n].T, W_hh[n].T], axis=0)       # (1024, 1536)
    a_re = _round_fp32r(
        a_full.reshape(KT, P, GT, P).transpose(2, 1, 0, 3)[_SLOT_TO_G]
    )                                                             # (GT, P, KT, P)
    u = _round_fp32r(
        np.concatenate(
            [x[:, n * BS : (n + 1) * BS].T, h[:, n * BS : (n + 1) * BS].T], axis=0
        )
    ).reshape(KT, P, B)
    brz8 = (b_ih[n, : 2 * BS] + b_hh[n, : 2 * BS]).reshape(8, P).T  # (P, 8)
    brz = np.ascontiguousarray(
        np.concatenate([brz8, -brz8[:, 4:8]], axis=1)
    )                                                             # (P, 12)
    bn = np.ascontiguousarray(
        np.concatenate(
            [b_ih[n, 2 * BS :].reshape(4, P).T, b_hh[n, 2 * BS :].reshape(4, P).T],
            axis=1,
        )
    )                                                             # (P, 8)
    return {"a": a_re, "u": u, "brz": brz, "bn": bn}


def kernel(x, h, W_ih, W_hh, b_ih, b_hh):
    global LAST_RESULTS
    x = np.asarray(x, dtype=np.float32)
    h = np.asarray(h, dtype=np.float32)
    W_ih = np.asarray(W_ih, dtype=np.float32)
    W_hh = np.asarray(W_hh, dtype=np.float32)
    b_ih = np.asarray(b_ih, dtype=np.float32)
    b_hh = np.asarray(b_hh, dtype=np.float32)

    if "nc" not in _cache:
        _cache["nc"] = _build_nc()
    nc = _cache["nc"]

    in_maps = [
        _prep_core_inputs(x, h, W_ih, W_hh, b_ih, b_hh, n) for n in range(NB)
    ]
    trace = os.environ.get("BASS_KERNEL_TRACE") == "1"
    res = run_bass_kernel_spmd(nc, in_maps, list(range(NB)), trace=trace)
    LAST_RESULTS = res
    return np.concatenate([res.results[n]["o"].T for n in range(NB)], axis=1)



# revision 2
# speedup vs baseline: 1.1850x; 1.1850x over previous
"""Block-diagonal GRU cell on 8 TRN2 NeuronCores — one block per core.

Math per block n (torch GRUCell):
  gi = x_n @ W_ih[n].T + b_ih[n]        (B, 3*BS)
  gh = h_n @ W_hh[n].T + b_hh[n]
  r = sigmoid(gi_r + gh_r); z = sigmoid(gi_z + gh_z)
  ng = tanh(gi_n + r * gh_n)
  h' = ng + z * (h_n - ng)

On-chip layout (per core): everything transposed on host so the
contraction (feature) dim is the SBUF partition dim and gates land on
PSUM partitions — biases then apply as per-partition ACT/DVE operands.
  A  = [W_ih[n].T ; W_hh[n].T]  -> (1024 feat, 1536 gates) in bf16,
       blocked per 128-gate column group, dram laid out partition-major
       so every slot-range load is one big contiguous-per-partition DMA.
  U  = [x_n.T ; h_n.T]          -> (1024 feat, 1024 batch) bf16
  out = h'.T                    -> (512, 1024) bf16, un-transposed and
       upcast on host.
All matmuls run in bf16 (full-rate PE, cheap LDWEIGHTS, half the HBM
traffic of fp32r). r/z gates accumulate x- and h-matmuls into one PSUM
bank (8 k-steps); the n gate keeps i_n / h_n in separate banks.
Combine avoids 1-z entirely: h' = nt - z*nt + z*h, with bf16
SBUF-resident DVE ops (2x dve mode) for everything not reading PSUM.
Loads are spread over three DGE queues (A on Sync, U on GpSimd SWDGE,
biases on Scalar) so trigger issue overhead doesn't serialize the fill;
short 128-wide PE warm-up matmuls bridge the fill for the p-state ramp.
"""

import os
import sys

import numpy as np

try:
    import concourse.bass as bass
except ImportError:  # fresh grading dir: fall back to the repo checkout
    sys.path.insert(0, "/opt/trn_rl_repo")
    import concourse.bass as bass

import concourse.mybir as mybir
import concourse.tile as tile
from concourse import bacc
from concourse.bass import ts
from concourse.bass_utils import run_bass_kernel_spmd

B = 1024            # batch
NB = 8              # blocks == cores
BS = 512            # hidden block size
G3 = 3 * BS         # gates per block (r, z, n)
KF = 1024           # contraction feats per core: 512 input + 512 hidden
P = 128
KT = KF // P        # 8 k-tiles
GT = G3 // P        # 12 gate column groups: 0-3 r, 4-7 z, 8-11 n
NBC = 2             # batch chunks
BC = B // NBC       # 512 (one PSUM bank of fp32)

F32 = mybir.dt.float32
BF16 = mybir.dt.bfloat16
AFT = mybir.ActivationFunctionType
ALU = mybir.AluOpType

_cache: dict = {}
LAST_RESULTS = None  # BassKernelResults of the most recent run (for test.py)


def _build_nc():
    nc = bacc.Bacc("TRN2", target_bir_lowering=False, debug=False, num_devices=NB)
    a_d = nc.dram_tensor("a", [P, GT * KT, P], BF16, kind="ExternalInput").ap()
    u_d = nc.dram_tensor("u", [P, KT, B], BF16, kind="ExternalInput").ap()
    brz_d = nc.dram_tensor("brz", [P, 12], F32, kind="ExternalInput").ap()
    bn_d = nc.dram_tensor("bn", [P, 8], F32, kind="ExternalInput").ap()
    o_d = nc.dram_tensor("o", [BS, B], BF16, kind="ExternalOutput").ap()

    with tile.TileContext(nc) as tc:
        with (
            tc.tile_pool(name="persist", bufs=1) as persist,
            tc.tile_pool(name="tmp", bufs=3) as tmp,
            tc.tile_pool(name="outp", bufs=4) as outp,
            tc.tile_pool(name="psum", bufs=8, space="PSUM") as psum,
        ):
            # tiny bias loads on the Scalar engine's queue, off the bulk paths
            brz_sb = persist.tile([P, 12], F32, name="brz_sb")
            nc.scalar.dma_start(brz_sb[:], brz_d[:])
            bn_sb = persist.tile([P, 8], F32, name="bn_sb")
            nc.scalar.dma_start(bn_sb[:], bn_d[:])

            U = persist.tile([P, KT, B], BF16, name="U")
            A = persist.tile([P, GT * KT, P], BF16, name="A")

            # PE warm-up scratch first on gpsimd so it's ready immediately
            wsb = persist.tile([P, P], BF16, name="wsb")
            nc.gpsimd.memset(wsb[:], 0.0)

            # Bulk loads, in consumption order per queue:
            #   Sync   : A slot ranges (r0, z0, n0, j=1, j=2+3)
            #   GpSimd : U (bc0 k0-3, bc0 k4-7, bc1 all)
            def load_a(s0, s1):
                nc.sync.dma_start(
                    A[:, s0 * KT : s1 * KT, :], a_d[:, s0 * KT : s1 * KT, :]
                )

            def load_u(k0, k1, bc):
                nc.gpsimd.dma_start(
                    U[:, k0:k1, ts(bc, BC)], u_d[:, k0:k1, ts(bc, BC)]
                )

            load_a(0, 1)
            load_u(0, 4, 0)
            load_a(1, 2)
            load_u(4, 8, 0)
            load_a(2, 3)
            load_u(0, 8, 1)
            load_a(3, 6)
            load_a(6, 12)

            # PE warm-up: short 128-wide matmuls bridge the DMA fill so the
            # HAM activity window is hot when real matmuls start
            wps = psum.tile([P, BC], F32, name="wps", tag="ps")
            for _ in range(16):
                nc.tensor.matmul(wps[:, :P], wsb[:], wsb[:], start=True, stop=True)

            # logical gate group -> A slot: slot 3j=r_j (g=j), 3j+1=z_j
            # (g=4+j), 3j+2=n_j (g=8+j)
            def slot_of(g):
                j, kind = g % 4, g // 4
                return 3 * j + kind

            def lhsT(g, k):
                return A[:, slot_of(g) * KT + k, :]

            # persistent per row-block j: r gate, z gate, zh = z*h (bf16)
            r_t = [persist.tile([P, B], BF16, name=f"r{j}") for j in range(4)]
            z_t = [persist.tile([P, B], BF16, name=f"z{j}") for j in range(4)]
            zh = [persist.tile([P, B], BF16, name=f"zh{j}") for j in range(4)]

            def mm_group(g, c0, w, k0, k1):
                ps = psum.tile([P, w], F32, name="ps", tag="ps")
                for k in range(k0, k1):
                    nc.tensor.matmul(
                        ps[:],
                        lhsT(g, k),
                        U[:, k, c0 : c0 + w],
                        start=(k == k0),
                        stop=(k == k1 - 1),
                    )
                return ps

            def combine(j, c0, w, ps_i, ps_h, sl, omz=None):
                # ng = tanh(i_n + b_in + r*(h_n + b_hn))
                # h' = ng - z*ng + z*h  (or omz*ng + zh on the tail path)
                t = tmp.tile([P, w], F32, name="t", tag="t")
                nc.vector.scalar_tensor_tensor(
                    t[:], ps_h[:, sl], bn_sb[:, 4 + j : 5 + j],
                    r_t[j][:, c0 : c0 + w], ALU.add, ALU.mult,
                )
                t2 = tmp.tile([P, w], BF16, name="t2", tag="t2")
                nc.vector.tensor_add(t2[:], t[:], ps_i[:, sl])
                nt = tmp.tile([P, w], BF16, name="nt", tag="nt")
                nc.scalar.activation(nt[:], t2[:], AFT.Tanh, bias=bn_sb[:, j : j + 1])
                m = tmp.tile([P, w], BF16, name="m", tag="m")
                if omz is None:
                    zn = tmp.tile([P, w], BF16, name="zn", tag="zn")
                    nc.vector.tensor_mul(zn[:], z_t[j][:, c0 : c0 + w], nt[:])
                    nc.vector.tensor_sub(m[:], nt[:], zn[:])
                else:
                    nc.vector.tensor_mul(m[:], omz[:, sl], nt[:])
                o_t = outp.tile([P, w], BF16, name="o_t", tag="o_t")
                nc.vector.tensor_add(o_t[:], m[:], zh[j][:, c0 : c0 + w])
                nc.sync.dma_start(o_d[ts(j, P), c0 : c0 + w], o_t[:])

            for bc in range(NBC):
                for j in range(4):
                    last = bc == NBC - 1 and j == 3
                    ps_r = mm_group(j, bc * BC, BC, 0, KT)
                    nc.scalar.activation(
                        r_t[j][:, ts(bc, BC)], ps_r[:], AFT.Sigmoid,
                        bias=brz_sb[:, j : j + 1],
                    )
                    ps_z = mm_group(4 + j, bc * BC, BC, 0, KT)
                    nc.scalar.activation(
                        z_t[j][:, ts(bc, BC)], ps_z[:], AFT.Sigmoid,
                        bias=brz_sb[:, 4 + j : 5 + j],
                    )
                    omz = None
                    if last:
                        # tail path: precompute 1-z off the critical chain so
                        # the post-matmul tail is one op shorter
                        # 1 - sigmoid(x) == sigmoid(-x); col 8+j holds -b_z
                        omz = tmp.tile([P, BC], BF16, name="omz", tag="omz")
                        nc.scalar.activation(
                            omz[:], ps_z[:], AFT.Sigmoid,
                            bias=brz_sb[:, 8 + j : 9 + j], scale=-1.0,
                        )
                    nc.vector.tensor_mul(
                        zh[j][:, ts(bc, BC)], z_t[j][:, ts(bc, BC)],
                        U[:, 4 + j, ts(bc, BC)],
                    )
                    if last:
                        # final group: half-width n-gate psums so only one
                        # short 256-wide combine chain trails the last matmul
                        HW_ = BC // 2
                        for s in range(2):
                            c0 = bc * BC + s * HW_
                            ps_h = mm_group(8 + j, c0, HW_, 4, KT)
                            ps_i = mm_group(8 + j, c0, HW_, 0, 4)
                            combine(
                                j, c0, HW_, ps_i, ps_h, slice(0, HW_),
                                omz=omz[:, s * HW_ : (s + 1) * HW_],
                            )
                    else:
                        ps_h = mm_group(8 + j, bc * BC, BC, 4, KT)
                        ps_i = mm_group(8 + j, bc * BC, BC, 0, 4)
                        combine(j, bc * BC, BC, ps_i, ps_h, slice(0, BC))

    nc.compile()
    return nc


_SLOT_TO_G = [g for j in range(4) for g in (j, 4 + j, 8 + j)]


def _prep_core_inputs(x16, h16, W_ih16, W_hh16, b_ih, b_hh, n):
    bf16 = x16.dtype
    a_full = np.concatenate([W_ih16[n].T, W_hh16[n].T], axis=0)      # (1024, 1536)
    a_re = np.ascontiguousarray(
        a_full.reshape(KT, P, GT, P).transpose(2, 1, 0, 3)[_SLOT_TO_G]
        .transpose(1, 0, 2, 3)
        .reshape(P, GT * KT, P)
    )                                                                # (P, GT*KT, P)
    u = np.ascontiguousarray(
        np.concatenate(
            [x16[:, n * BS : (n + 1) * BS].T, h16[:, n * BS : (n + 1) * BS].T],
            axis=0,
        ).reshape(KT, P, B).transpose(1, 0, 2)
    )                                                                # (P, KT, B)
    brz8 = (b_ih[n, : 2 * BS] + b_hh[n, : 2 * BS]).reshape(8, P).T   # (P, 8)
    brz = np.ascontiguousarray(
        np.concatenate([brz8, -brz8[:, 4:8]], axis=1)
    )                                                                # (P, 12)
    bn = np.ascontiguousarray(
        np.concatenate(
            [b_ih[n, 2 * BS :].reshape(4, P).T, b_hh[n, 2 * BS :].reshape(4, P).T],
            axis=1,
        )
    )                                                                # (P, 8)
    return {"a": a_re, "u": u, "brz": brz, "bn": bn}


def kernel(x, h, W_ih, W_hh, b_ih, b_hh):
    global LAST_RESULTS
    import ml_dtypes

    bf16 = np.dtype(ml_dtypes.bfloat16)
    x16 = np.asarray(x, dtype=np.float32).astype(bf16)
    h16 = np.asarray(h, dtype=np.float32).astype(bf16)
    W_ih16 = np.asarray(W_ih, dtype=np.float32).astype(bf16)
    W_hh16 = np.asarray(W_hh, dtype=np.float32).astype(bf16)
    b_ih = np.asarray(b_ih, dtype=np.float32)
    b_hh = np.asarray(b_hh, dtype=np.float32)

    if "nc" not in _cache:
        _cache["nc"] = _build_nc()
    nc = _cache["nc"]

    in_maps = [
        _prep_core_inputs(x16, h16, W_ih16, W_hh16, b_ih, b_hh, n)
        for n in range(NB)
    ]
    trace = os.environ.get("BASS_KERNEL_TRACE") == "1"
    res = run_bass_kernel_spmd(nc, in_maps, list(range(NB)), trace=trace)
    LAST_RESULTS = res
    return np.concatenate(
        [res.results[n]["o"].astype(np.float32).T for n in range(NB)], axis=1
    )
